# revision 17
# baseline (speedup 1.0000x reference)
"""Bounded compositional energy kernel for 8 Trainium2 NeuronCores.

Math (per batch row x = [state | action], IN=1280):
  h_c   = x @ W1[c] + b1[c]                  (C=8 components, H=1024)
  hn_c  = LN(h_c) * gamma[c] + beta[c]
  g_c   = gelu_erf(hn_c)
  comp_c = g_c . W2[c] + b2[c]
  w     = softmax(x @ Ww + bw)
  energy = sum_c comp_c * w_c

Sharding: data-parallel over batch, 2048 rows/core, replicated weights.
Per core the 2048 rows are processed in 2 passes of 1024 rows (8 batch
tiles of 128) so that x^T (stationary matmul operand), the
double-buffered W1[c] stream, and the epilogue staging all fit in SBUF.

Main GEMM runs as float32r (full PE rate at N=512). LayerNorm stats via
bn_stats/bn_aggr on DVE; sqrt(var+eps) is batched per-c on ACT (one
table switch pair per c instead of per tile — ACT tables cost 1.28us to
swap); the normalize affine is folded into the GELU activation's
per-partition scale/bias when gamma/beta allow; the W2 dot + b2 bias is
one fused tensor_tensor_reduce.
"""

import os
import sys

for _p in ("/opt/trn_rl_repo", "/root/.axon_site/_ro/trn_rl_repo"):
    if os.path.isdir(_p) and _p not in sys.path:
        sys.path.insert(0, _p)

import numpy as np

import concourse.bass as bass
import concourse.tile as tile
from concourse import bacc, mybir
from concourse.bass_utils import run_bass_kernel_spmd
from concourse.masks import make_identity

F32 = mybir.dt.float32
F32R = mybir.dt.float32r

N_CORES = 8
B = 16384
B_LOCAL = B // N_CORES      # 2048
STATE_DIM = 1024
ACTION_DIM = 256
IN_DIM = STATE_DIM + ACTION_DIM   # 1280
KC = IN_DIM // 128                # 10 contraction chunks
C = 8
H = 1024
JH = H // 512                     # 2 psum-bank halves of the H dim
EPS = 1e-5

N_PASS = 2
B_PASS = B_LOCAL // N_PASS        # 1024 rows per pass
NBT = B_PASS // 128               # 8 batch tiles per pass
SQG = 4                           # batch tiles per batched-sqrt group


def _build(gamma_trivial: bool, use_f32r: bool = True):
    """Emit the single-core program (SPMD across 8 cores)."""
    MMDT = F32R if use_f32r else F32
    nc = bacc.Bacc("TRN2", target_bir_lowering=False, debug=False,
                   num_devices=N_CORES)

    d_states = nc.dram_tensor("states", [B_LOCAL, STATE_DIM], F32,
                              kind="ExternalInput").ap()
    d_actions = nc.dram_tensor("actions", [B_LOCAL, ACTION_DIM], F32,
                               kind="ExternalInput").ap()
    d_w1 = nc.dram_tensor("W1", [C, IN_DIM, H], F32, kind="ExternalInput").ap()
    d_b1 = nc.dram_tensor("b1", [C, H], F32, kind="ExternalInput").ap()
    if not gamma_trivial:
        d_gamma = nc.dram_tensor("gamma", [C, H], F32,
                                 kind="ExternalInput").ap()
        d_beta = nc.dram_tensor("beta", [C, H], F32,
                                kind="ExternalInput").ap()
    d_w2 = nc.dram_tensor("W2", [C, H], F32, kind="ExternalInput").ap()
    d_b2 = nc.dram_tensor("b2", [C], F32, kind="ExternalInput").ap()
    d_ww = nc.dram_tensor("Ww", [IN_DIM, C], F32, kind="ExternalInput").ap()
    d_bw = nc.dram_tensor("bw", [C], F32, kind="ExternalInput").ap()
    d_out = nc.dram_tensor("energy", [B_LOCAL, 1], F32,
                           kind="ExternalOutput").ap()

    with tile.TileContext(nc) as tc:
        with (
            tc.tile_pool(name="singles", bufs=1) as singles,
            tc.tile_pool(name="xstage", bufs=2) as xstage,
            tc.tile_pool(name="xt", bufs=1) as xtp,
            tc.tile_pool(name="w1", bufs=2) as w1p,
            tc.tile_pool(name="bcast", bufs=2) as bcastp,
            tc.tile_pool(name="sstage", bufs=5) as sstage,
            tc.tile_pool(name="gstage", bufs=2) as gstage,
            tc.tile_pool(name="scrp", bufs=1) as scrp,
            tc.tile_pool(name="small", bufs=4) as small,
            tc.tile_pool(name="pt", bufs=2, space="PSUM") as pt,
            tc.tile_pool(name="ph", bufs=2, space="PSUM") as ph,
            tc.tile_pool(name="pl", bufs=2, space="PSUM") as pl,
        ):
            ident = singles.tile([128, 128], F32)
            make_identity(nc, ident)
            eps_t = singles.tile([128, 1], F32)
            nc.vector.memset(eps_t, EPS)

            # weight-net Ww -> [128, KC, C], b2/bw broadcast [128, C]
            ww_sb = singles.tile([128, KC, C], MMDT)
            nc.sync.dma_start(
                out=ww_sb,
                in_=d_ww.rearrange("(k p) c -> p k c", p=128).bitcast(MMDT))
            b2_sb = singles.tile([128, C], F32)
            nc.sync.dma_start(out=b2_sb, in_=d_b2[None, :].to_broadcast([128, C]))
            bw_sb = singles.tile([128, C], F32)
            nc.sync.dma_start(out=bw_sb, in_=d_bw[None, :].to_broadcast([128, C]))

            # per-pass resident x^T: [128 (i in chunk), KC, B_PASS]
            for p in range(N_PASS):
                r0 = p * B_PASS
                xt = xtp.tile([128, KC, B_PASS], MMDT, tag="xt")

                logit_sb = small.tile([128, NBT, C], F32, tag="logit")
                comps = small.tile([128, NBT, C], F32, tag="comps")

                for bt in range(NBT):
                    xs = xstage.tile([128, IN_DIM], F32, tag="xs")
                    nc.sync.dma_start(
                        out=xs[:, 0:STATE_DIM],
                        in_=d_states[r0 + bt * 128: r0 + (bt + 1) * 128, :])
                    nc.sync.dma_start(
                        out=xs[:, STATE_DIM:IN_DIM],
                        in_=d_actions[r0 + bt * 128: r0 + (bt + 1) * 128, :])
                    for k in range(KC):
                        tp = pt.tile([128, 128], F32, tag="tp")
                        nc.tensor.transpose(
                            tp, xs[:, k * 128:(k + 1) * 128], ident)
                        nc.scalar.copy(
                            out=xt[:, k, bt * 128:(bt + 1) * 128], in_=tp)

                    # weight-net logits for this tile: x @ Ww (+ bw)
                    lp = pl.tile([128, C], F32, tag="lp")
                    for k in range(KC):
                        nc.tensor.matmul(
                            lp, xt[:, k, bt * 128:(bt + 1) * 128],
                            ww_sb[:, k, :], start=(k == 0), stop=(k == KC - 1))
                    nc.vector.tensor_add(
                        logit_sb[:, bt, :], lp, bw_sb)

                for c in range(C):
                    w1t = w1p.tile([128, KC, H], MMDT, tag="w1")
                    w1v = d_w1[c].rearrange("(k p) h -> p k h", p=128)
                    for k0 in range(0, KC, 3):
                        k1 = min(k0 + 3, KC)
                        nc.sync.dma_start(
                            out=w1t[:, k0:k1, :],
                            in_=w1v[:, k0:k1, :].bitcast(MMDT))

                    b1_b = bcastp.tile([128, H], F32, tag="b1b")
                    nc.sync.dma_start(
                        out=b1_b, in_=d_b1[c][None, :].to_broadcast([128, H]))
                    w2_b = bcastp.tile([128, H], F32, tag="w2b")
                    nc.sync.dma_start(
                        out=w2_b, in_=d_w2[c][None, :].to_broadcast([128, H]))
                    if not gamma_trivial:
                        ga_b = bcastp.tile([128, H], F32, tag="gab")
                        nc.sync.dma_start(
                            out=ga_b,
                            in_=d_gamma[c][None, :].to_broadcast([128, H]))
                        be_b = bcastp.tile([128, H], F32, tag="beb")
                        nc.sync.dma_start(
                            out=be_b,
                            in_=d_beta[c][None, :].to_broadcast([128, H]))

                    for g0 in range(0, NBT, SQG):
                      bts = range(g0, min(g0 + SQG, NBT))
                      mvg = small.tile([128, SQG, 2], F32, tag="mvg")
                      s_tiles = {}
                      for bt in bts:
                        hp = ph.tile([128, H], F32, tag="hp")
                        for jh in range(JH):
                            js = slice(jh * 512, (jh + 1) * 512)
                            for k in range(KC):
                                nc.tensor.matmul(
                                    hp[:, js],
                                    xt[:, k, bt * 128:(bt + 1) * 128],
                                    w1t[:, k, js],
                                    start=(k == 0), stop=(k == KC - 1))
                        # drain psum + b1 bias add
                        s = sstage.tile([128, H], F32, tag="s")
                        nc.vector.tensor_add(s, hp, b1_b)
                        st = small.tile([128, 2, 6], F32, tag="st")
                        nc.vector.bn_stats(out=st[:, 0, :], in_=s[:, 0:512])
                        nc.vector.bn_stats(out=st[:, 1, :], in_=s[:, 512:H])
                        nc.vector.bn_aggr(out=mvg[:, bt - g0, :], in_=st)
                        s_tiles[bt] = s

                      # batched rstd for SQG tiles: one sqrt-table
                      # round-trip instead of one per tile.
                      std_g = small.tile([128, SQG], F32, tag="stdg")
                      nc.scalar.activation(
                          out=std_g, in_=mvg[:, :, 1],
                          func=mybir.ActivationFunctionType.Sqrt,
                          bias=eps_t, scale=1.0)
                      rstd_g = small.tile([128, SQG], F32, tag="rstdg")
                      nc.vector.reciprocal(rstd_g, std_g)
                      nmr_g = small.tile([128, SQG], F32, tag="nmrg")
                      nc.vector.scalar_tensor_tensor(
                          out=nmr_g, in0=mvg[:, :, 0], scalar=-1.0,
                          in1=rstd_g,
                          op0=mybir.AluOpType.mult, op1=mybir.AluOpType.mult)

                      for bt in bts:
                        s = s_tiles[bt]
                        gi = bt - g0
                        g = gstage.tile([128, H], F32, tag="g")
                        if gamma_trivial:
                            # gelu(LN(s)) with the normalize affine folded
                            # into the activation's per-partition scale/bias
                            nc.scalar.activation(
                                out=g, in_=s,
                                func=mybir.ActivationFunctionType.Gelu,
                                bias=nmr_g[:, gi:gi + 1],
                                scale=rstd_g[:, gi:gi + 1])
                        else:
                            hn = gstage.tile([128, H], F32, tag="hn")
                            nc.scalar.activation(
                                out=hn, in_=s,
                                func=mybir.ActivationFunctionType.Identity,
                                bias=nmr_g[:, gi:gi + 1],
                                scale=rstd_g[:, gi:gi + 1])
                            nc.vector.tensor_mul(hn, hn, ga_b)
                            nc.vector.tensor_add(hn, hn, be_b)
                            nc.scalar.activation(
                                out=g, in_=hn,
                                func=mybir.ActivationFunctionType.Gelu)
                        # comps[:, bt, c] = sum_j g * W2[c]
                        # (InstTensorTensorReduce wedges TRN2 here; the
                        # scalar_tensor_tensor accumulate path is the one
                        # that runs. b2 is added after the c loop.)
                        scr = scrp.tile([128, H], F32, tag="scr")
                        nc.vector.scalar_tensor_tensor(
                            out=scr, in0=g, scalar=1.0, in1=w2_b,
                            op0=mybir.AluOpType.mult,
                            op1=mybir.AluOpType.mult,
                            accum_out=comps[:, bt, c:c + 1])

                # softmax over components + energy combine, batched per pass
                nc.vector.tensor_add(
                    comps, comps,
                    b2_sb[:, None, :].to_broadcast([128, NBT, C]))
                ener = small.tile([128, NBT], F32, tag="ener")
                nmax = small.tile([128, NBT], F32, tag="nmax")
                esum = small.tile([128, NBT], F32, tag="esum")
                rsum = small.tile([128, NBT], F32, tag="rsum")
                ew = small.tile([128, NBT, C], F32, tag="ew")
                for bt in range(NBT):
                    nc.vector.reduce_max(
                        nmax[:, bt:bt + 1], logit_sb[:, bt, :],
                        axis=mybir.AxisListType.X, negate=True)
                for bt in range(NBT):
                    nc.scalar.activation(
                        out=ew[:, bt, :], in_=logit_sb[:, bt, :],
                        func=mybir.ActivationFunctionType.Exp,
                        bias=nmax[:, bt:bt + 1], scale=1.0,
                        accum_out=esum[:, bt:bt + 1])
                nc.vector.reciprocal(rsum, esum)
                for bt in range(NBT):
                    scr8 = small.tile([128, C], F32, tag="scr8")
                    nc.vector.scalar_tensor_tensor(
                        out=scr8, in0=comps[:, bt, :], scalar=1.0,
                        in1=ew[:, bt, :],
                        op0=mybir.AluOpType.mult, op1=mybir.AluOpType.mult,
                        accum_out=ener[:, bt:bt + 1])
                enorm = small.tile([128, NBT], F32, tag="enorm")
                nc.vector.tensor_mul(enorm, ener, rsum)
                nc.sync.dma_start(
                    out=d_out[r0:r0 + B_PASS, 0].rearrange(
                        "(t p) -> p t", p=128),
                    in_=enorm)

    nc.compile()
    return nc


_BUILT = {}

# test-harness hooks; the grading path leaves these at their defaults
TRACE = False
LAST_RESULT = None


def _get_nc(gamma_trivial: bool, use_f32r: bool = True):
    key = (gamma_trivial, use_f32r)
    if key not in _BUILT:
        _BUILT[key] = _build(gamma_trivial, use_f32r)
    return _BUILT[key]


def kernel(states, actions, W1, b1, gamma, beta, W2, b2, Ww, bw):
    states = np.ascontiguousarray(states, dtype=np.float32)
    actions = np.ascontiguousarray(actions, dtype=np.float32)
    W1 = np.ascontiguousarray(W1, dtype=np.float32)
    b1 = np.ascontiguousarray(b1, dtype=np.float32)
    gamma = np.ascontiguousarray(gamma, dtype=np.float32)
    beta = np.ascontiguousarray(beta, dtype=np.float32)
    W2 = np.ascontiguousarray(W2, dtype=np.float32)
    b2 = np.ascontiguousarray(b2, dtype=np.float32)
    Ww = np.ascontiguousarray(Ww, dtype=np.float32)
    bw = np.ascontiguousarray(bw, dtype=np.float32)

    gamma_trivial = bool(np.all(gamma == 1.0) and np.all(beta == 0.0))
    nc = _get_nc(gamma_trivial)

    shared = {"W1": W1, "b1": b1, "gamma": gamma, "beta": beta,
              "W2": W2, "b2": b2, "Ww": Ww, "bw": bw}
    in_maps = []
    for i in range(N_CORES):
        m = dict(shared)
        m["states"] = states[i * B_LOCAL:(i + 1) * B_LOCAL]
        m["actions"] = actions[i * B_LOCAL:(i + 1) * B_LOCAL]
        in_maps.append(m)

    res = run_bass_kernel_spmd(nc, in_maps, core_ids=list(range(N_CORES)),
                               trace=TRACE)
    global LAST_RESULT
    LAST_RESULT = res
    return np.concatenate(
        [res.results[i]["energy"] for i in range(N_CORES)], axis=0)


# revision 21
# speedup vs baseline: 1.5330x; 1.5330x over previous
"""Bounded compositional energy kernel for 8 Trainium2 NeuronCores.

Math (per batch row x = [state | action], IN=1280):
  h_c   = x @ W1[c] + b1[c]                  (C=8 components, H=1024)
  hn_c  = LN(h_c) * gamma[c] + beta[c]
  g_c   = gelu_erf(hn_c)
  comp_c = g_c . W2[c] + b2[c]
  w     = softmax(x @ Ww + bw)
  energy = sum_c comp_c * w_c

Sharding: data-parallel over batch, 2048 rows/core, replicated weights.
Per core the 2048 rows are processed in 2 passes of 1024 rows (8 batch
tiles of 128) so that x^T (stationary matmul operand), the
double-buffered W1[c] stream, and the epilogue staging all fit in SBUF.

Main GEMM runs as float32r (full PE rate at N=512). LayerNorm stats via
bn_stats/bn_aggr on DVE; sqrt(var+eps) is batched per-c on ACT (one
table switch pair per c instead of per tile — ACT tables cost 1.28us to
swap); the normalize affine is folded into the GELU activation's
per-partition scale/bias when gamma/beta allow; the W2 dot + b2 bias is
one fused tensor_tensor_reduce.
"""

import os
import sys

for _p in ("/opt/trn_rl_repo", "/root/.axon_site/_ro/trn_rl_repo"):
    if os.path.isdir(_p) and _p not in sys.path:
        sys.path.insert(0, _p)

import numpy as np

import concourse.bass as bass
import concourse.tile as tile
from concourse import bacc, mybir
from concourse.bass_utils import run_bass_kernel_spmd
from concourse.masks import make_identity

F32 = mybir.dt.float32
F32R = mybir.dt.float32r

N_CORES = 8
B = 16384
B_LOCAL = B // N_CORES      # 2048
STATE_DIM = 1024
ACTION_DIM = 256
IN_DIM = STATE_DIM + ACTION_DIM   # 1280
KC = IN_DIM // 128                # 10 contraction chunks
C = 8
H = 1024
JH = H // 512                     # 2 psum-bank halves of the H dim
EPS = 1e-5

N_PASS = 2
B_PASS = B_LOCAL // N_PASS        # 1024 rows per pass
NBT = B_PASS // 128               # 8 batch tiles per pass
SQG = 4                           # batch tiles per batched-sqrt group


def _build(gamma_trivial: bool, use_f32r: bool = True):
    """Emit the single-core program (SPMD across 8 cores)."""
    MMDT = F32R if use_f32r else F32
    nc = bacc.Bacc("TRN2", target_bir_lowering=False, debug=False,
                   num_devices=N_CORES)

    d_states = nc.dram_tensor("states", [B_LOCAL, STATE_DIM], F32,
                              kind="ExternalInput").ap()
    d_actions = nc.dram_tensor("actions", [B_LOCAL, ACTION_DIM], F32,
                               kind="ExternalInput").ap()
    d_w1 = nc.dram_tensor("W1", [C, IN_DIM, H], F32, kind="ExternalInput").ap()
    d_b1 = nc.dram_tensor("b1", [C, H], F32, kind="ExternalInput").ap()
    if not gamma_trivial:
        d_gamma = nc.dram_tensor("gamma", [C, H], F32,
                                 kind="ExternalInput").ap()
        d_beta = nc.dram_tensor("beta", [C, H], F32,
                                kind="ExternalInput").ap()
    d_w2 = nc.dram_tensor("W2", [C, H], F32, kind="ExternalInput").ap()
    d_b2 = nc.dram_tensor("b2", [C], F32, kind="ExternalInput").ap()
    d_ww = nc.dram_tensor("Ww", [IN_DIM, C], F32, kind="ExternalInput").ap()
    d_bw = nc.dram_tensor("bw", [C], F32, kind="ExternalInput").ap()
    d_out = nc.dram_tensor("energy", [B_LOCAL, 1], F32,
                           kind="ExternalOutput").ap()

    with tile.TileContext(nc) as tc:
        with (
            tc.tile_pool(name="singles", bufs=1) as singles,
            tc.tile_pool(name="xstage", bufs=2) as xstage,
            tc.tile_pool(name="xt", bufs=1) as xtp,
            tc.tile_pool(name="w1", bufs=2) as w1p,
            tc.tile_pool(name="bcast", bufs=2) as bcastp,
            tc.tile_pool(name="sstage", bufs=5) as sstage,
            tc.tile_pool(name="gstage", bufs=2) as gstage,
            tc.tile_pool(name="scrp", bufs=1) as scrp,
            tc.tile_pool(name="small", bufs=4) as small,
            tc.tile_pool(name="pt", bufs=2, space="PSUM") as pt,
            tc.tile_pool(name="ph", bufs=2, space="PSUM") as ph,
            tc.tile_pool(name="pl", bufs=2, space="PSUM") as pl,
        ):
            ident = singles.tile([128, 128], F32)
            make_identity(nc, ident)
            eps_t = singles.tile([128, 1], F32)
            nc.vector.memset(eps_t, EPS)

            # weight-net Ww -> [128, KC, C], b2/bw broadcast [128, C]
            ww_sb = singles.tile([128, KC, C], MMDT)
            nc.sync.dma_start(
                out=ww_sb,
                in_=d_ww.rearrange("(k p) c -> p k c", p=128).bitcast(MMDT))
            b2_sb = singles.tile([128, C], F32)
            nc.sync.dma_start(out=b2_sb, in_=d_b2[None, :].to_broadcast([128, C]))
            bw_sb = singles.tile([128, C], F32)
            nc.sync.dma_start(out=bw_sb, in_=d_bw[None, :].to_broadcast([128, C]))

            # per-pass resident x^T: [128 (i in chunk), KC, B_PASS]
            for p in range(N_PASS):
                r0 = p * B_PASS
                xt = xtp.tile([128, KC, B_PASS], MMDT, tag="xt")

                logit_sb = small.tile([128, NBT, C], F32, tag="logit")
                comps = small.tile([128, NBT, C], F32, tag="comps")

                for bt in range(NBT):
                    xs = xstage.tile([128, IN_DIM], F32, tag="xs")
                    nc.sync.dma_start(
                        out=xs[:, 0:STATE_DIM],
                        in_=d_states[r0 + bt * 128: r0 + (bt + 1) * 128, :])
                    nc.sync.dma_start(
                        out=xs[:, STATE_DIM:IN_DIM],
                        in_=d_actions[r0 + bt * 128: r0 + (bt + 1) * 128, :])
                    for k in range(KC):
                        tp = pt.tile([128, 128], F32, tag="tp")
                        nc.tensor.transpose(
                            tp, xs[:, k * 128:(k + 1) * 128], ident)
                        nc.vector.tensor_copy(
                            out=xt[:, k, bt * 128:(bt + 1) * 128], in_=tp)

                    # weight-net logits for this tile: x @ Ww (+ bw)
                    lp = pl.tile([128, C], F32, tag="lp")
                    for k in range(KC):
                        nc.tensor.matmul(
                            lp, xt[:, k, bt * 128:(bt + 1) * 128],
                            ww_sb[:, k, :], start=(k == 0), stop=(k == KC - 1))
                    nc.vector.tensor_add(
                        logit_sb[:, bt, :], lp, bw_sb)

                # softmax weights up front — logits only need x, so the
                # exp-table load happens once here, before the gelu/sqrt
                # tables take over the c loop; also shortens the pass tail
                nmax = small.tile([128, NBT], F32, tag="nmax")
                esum = small.tile([128, NBT], F32, tag="esum")
                rsum = small.tile([128, NBT], F32, tag="rsum")
                ew = small.tile([128, NBT, C], F32, tag="ew")
                for bt in range(NBT):
                    nc.vector.reduce_max(
                        nmax[:, bt:bt + 1], logit_sb[:, bt, :],
                        axis=mybir.AxisListType.X, negate=True)
                for bt in range(NBT):
                    nc.scalar.activation(
                        out=ew[:, bt, :], in_=logit_sb[:, bt, :],
                        func=mybir.ActivationFunctionType.Exp,
                        bias=nmax[:, bt:bt + 1], scale=1.0,
                        accum_out=esum[:, bt:bt + 1])
                nc.vector.reciprocal(rsum, esum)

                for c in range(C):
                    w1t = w1p.tile([128, KC, H], MMDT, tag="w1")
                    w1v = d_w1[c].rearrange("(k p) h -> p k h", p=128)
                    for k0 in range(0, KC, 3):
                        k1 = min(k0 + 3, KC)
                        nc.sync.dma_start(
                            out=w1t[:, k0:k1, :],
                            in_=w1v[:, k0:k1, :].bitcast(MMDT))

                    b1_b = bcastp.tile([128, H], F32, tag="b1b")
                    nc.sync.dma_start(
                        out=b1_b, in_=d_b1[c][None, :].to_broadcast([128, H]))
                    w2_b = bcastp.tile([128, H], F32, tag="w2b")
                    nc.sync.dma_start(
                        out=w2_b, in_=d_w2[c][None, :].to_broadcast([128, H]))
                    if not gamma_trivial:
                        ga_b = bcastp.tile([128, H], F32, tag="gab")
                        nc.sync.dma_start(
                            out=ga_b,
                            in_=d_gamma[c][None, :].to_broadcast([128, H]))
                        be_b = bcastp.tile([128, H], F32, tag="beb")
                        nc.sync.dma_start(
                            out=be_b,
                            in_=d_beta[c][None, :].to_broadcast([128, H]))

                    for g0 in range(0, NBT, SQG):
                      bts = range(g0, min(g0 + SQG, NBT))
                      sums_g = small.tile([128, SQG], F32, tag="sumsg")
                      sqs_g = small.tile([128, SQG], F32, tag="sqsg")
                      s_tiles = {}
                      for bt in bts:
                        gi = bt - g0
                        hp = ph.tile([128, H], F32, tag="hp")
                        for jh in range(JH):
                            js = slice(jh * 512, (jh + 1) * 512)
                            for k in range(KC):
                                nc.tensor.matmul(
                                    hp[:, js],
                                    xt[:, k, bt * 128:(bt + 1) * 128],
                                    w1t[:, k, js],
                                    start=(k == 0), stop=(k == KC - 1))
                        # drain psum + b1 add; row-sum rides along free
                        s = sstage.tile([128, H], F32, tag="s")
                        nc.vector.scalar_tensor_tensor(
                            out=s, in0=hp, scalar=1.0, in1=b1_b,
                            op0=mybir.AluOpType.mult,
                            op1=mybir.AluOpType.add,
                            accum_out=sums_g[:, gi:gi + 1])
                        # row sum of squares on ACT (square is in every
                        # activation table — no table switch)
                        scra = gstage.tile([128, H], F32, tag="scra")
                        nc.scalar.activation(
                            out=scra, in_=s,
                            func=mybir.ActivationFunctionType.Square,
                            accum_out=sqs_g[:, gi:gi + 1])
                        s_tiles[bt] = s

                      # batched LN stats for SQG tiles: mean/var from the
                      # accumulated sums, one sqrt-table round-trip.
                      mu_g = small.tile([128, SQG], F32, tag="mug")
                      nc.vector.tensor_scalar_mul(mu_g, sums_g, 1.0 / H)
                      mu2_g = small.tile([128, SQG], F32, tag="mu2g")
                      nc.vector.tensor_mul(mu2_g, mu_g, mu_g)
                      var_g = small.tile([128, SQG], F32, tag="varg")
                      nc.vector.scalar_tensor_tensor(
                          out=var_g, in0=sqs_g, scalar=1.0 / H, in1=mu2_g,
                          op0=mybir.AluOpType.mult,
                          op1=mybir.AluOpType.subtract)
                      std_g = small.tile([128, SQG], F32, tag="stdg")
                      nc.scalar.activation(
                          out=std_g, in_=var_g,
                          func=mybir.ActivationFunctionType.Sqrt,
                          bias=eps_t, scale=1.0)
                      rstd_g = small.tile([128, SQG], F32, tag="rstdg")
                      nc.vector.reciprocal(rstd_g, std_g)
                      nmr_g = small.tile([128, SQG], F32, tag="nmrg")
                      nc.vector.scalar_tensor_tensor(
                          out=nmr_g, in0=mu_g, scalar=-1.0,
                          in1=rstd_g,
                          op0=mybir.AluOpType.mult, op1=mybir.AluOpType.mult)

                      for bt in bts:
                        s = s_tiles[bt]
                        gi = bt - g0
                        g = gstage.tile([128, H], F32, tag="g")
                        if gamma_trivial:
                            # gelu(LN(s)) with the normalize affine folded
                            # into the activation's per-partition scale/bias
                            nc.scalar.activation(
                                out=g, in_=s,
                                func=mybir.ActivationFunctionType.Gelu,
                                bias=nmr_g[:, gi:gi + 1],
                                scale=rstd_g[:, gi:gi + 1])
                        else:
                            hn = gstage.tile([128, H], F32, tag="hn")
                            nc.scalar.activation(
                                out=hn, in_=s,
                                func=mybir.ActivationFunctionType.Identity,
                                bias=nmr_g[:, gi:gi + 1],
                                scale=rstd_g[:, gi:gi + 1])
                            nc.vector.tensor_mul(hn, hn, ga_b)
                            nc.vector.tensor_add(hn, hn, be_b)
                            nc.scalar.activation(
                                out=g, in_=hn,
                                func=mybir.ActivationFunctionType.Gelu)
                        # comps[:, bt, c] = sum_j g * W2[c]
                        # (InstTensorTensorReduce wedges TRN2 here; the
                        # scalar_tensor_tensor accumulate path is the one
                        # that runs. b2 is added after the c loop.)
                        scr = scrp.tile([128, H], F32, tag="scr")
                        nc.vector.scalar_tensor_tensor(
                            out=scr, in0=g, scalar=1.0, in1=w2_b,
                            op0=mybir.AluOpType.mult,
                            op1=mybir.AluOpType.mult,
                            accum_out=comps[:, bt, c:c + 1])

                # energy combine, batched per pass
                nc.vector.tensor_add(
                    comps, comps,
                    b2_sb[:, None, :].to_broadcast([128, NBT, C]))
                ener = small.tile([128, NBT], F32, tag="ener")
                for bt in range(NBT):
                    scr8 = small.tile([128, C], F32, tag="scr8")
                    nc.vector.scalar_tensor_tensor(
                        out=scr8, in0=comps[:, bt, :], scalar=1.0,
                        in1=ew[:, bt, :],
                        op0=mybir.AluOpType.mult, op1=mybir.AluOpType.mult,
                        accum_out=ener[:, bt:bt + 1])
                enorm = small.tile([128, NBT], F32, tag="enorm")
                nc.vector.tensor_mul(enorm, ener, rsum)
                nc.sync.dma_start(
                    out=d_out[r0:r0 + B_PASS, 0].rearrange(
                        "(t p) -> p t", p=128),
                    in_=enorm)

    nc.compile()
    return nc


_BUILT = {}

# test-harness hooks; the grading path leaves these at their defaults
TRACE = False
LAST_RESULT = None


def _get_nc(gamma_trivial: bool, use_f32r: bool = True):
    key = (gamma_trivial, use_f32r)
    if key not in _BUILT:
        _BUILT[key] = _build(gamma_trivial, use_f32r)
    return _BUILT[key]


def kernel(states, actions, W1, b1, gamma, beta, W2, b2, Ww, bw):
    states = np.ascontiguousarray(states, dtype=np.float32)
    actions = np.ascontiguousarray(actions, dtype=np.float32)
    W1 = np.ascontiguousarray(W1, dtype=np.float32)
    b1 = np.ascontiguousarray(b1, dtype=np.float32)
    gamma = np.ascontiguousarray(gamma, dtype=np.float32)
    beta = np.ascontiguousarray(beta, dtype=np.float32)
    W2 = np.ascontiguousarray(W2, dtype=np.float32)
    b2 = np.ascontiguousarray(b2, dtype=np.float32)
    Ww = np.ascontiguousarray(Ww, dtype=np.float32)
    bw = np.ascontiguousarray(bw, dtype=np.float32)

    gamma_trivial = bool(np.all(gamma == 1.0) and np.all(beta == 0.0))
    nc = _get_nc(gamma_trivial)

    shared = {"W1": W1, "b1": b1, "gamma": gamma, "beta": beta,
              "W2": W2, "b2": b2, "Ww": Ww, "bw": bw}
    in_maps = []
    for i in range(N_CORES):
        m = dict(shared)
        m["states"] = states[i * B_LOCAL:(i + 1) * B_LOCAL]
        m["actions"] = actions[i * B_LOCAL:(i + 1) * B_LOCAL]
        in_maps.append(m)

    res = run_bass_kernel_spmd(nc, in_maps, core_ids=list(range(N_CORES)),
                               trace=TRACE)
    global LAST_RESULT
    LAST_RESULT = res
    return np.concatenate(
        [res.results[i]["energy"] for i in range(N_CORES)], axis=0)
